# revision 21
# baseline (speedup 1.0000x reference)
"""Mixtral sparse MoE block on 8 Trainium2 NeuronCores (expert parallelism).

Contract: kernel(**inputs) takes FULL unsharded numpy inputs and returns the
FULL output, matching reference.reference() (a tuple: (out[B,S,H], router_logits[T,E])).

Strategy:
  - Router (logits, softmax, top-2, weight normalization) runs on host with
    jax-on-CPU, replicating the reference ops so expert selection is
    bit-identical to the reference.
  - Tokens are dispatched host-side to their selected experts; core e runs
    expert e's FFN (silu(x@w1.T)*(x@w3.T))@w2.T over its (padded) token set
    at fp32r (TF32) precision on the PE, scaled by the routing weight.
  - Host scatters the per-expert outputs back and sums (each token has
    exactly 2 contributions). Any tokens beyond the per-expert capacity are
    computed on host in fp32 (never triggers for balanced routing).
"""

import os
import sys
import types

import numpy as np

for _p in ("/opt/trn_rl_repo",):
    if _p not in sys.path and os.path.isdir(_p):
        sys.path.append(_p)

B, S, H, F, E, TOPK = 4, 2048, 1024, 2048, 8, 2
T = B * S
C = 2048  # per-expert token capacity (multiple of 512)
KCH = H // 128   # 8  contraction chunks for stage 1
MCH = F // 128   # 16 F chunks
PHASES = ((0, (512, 512)), (1024, (512, 512)))  # (start, block lens)

_COMPILED = {}


def _round_tf32(a: np.ndarray) -> np.ndarray:
    """Round fp32 -> tf32 (10-bit mantissa), RNE, as fp32 storage."""
    u = np.ascontiguousarray(a, dtype=np.float32).view(np.uint32)
    u = (u + np.uint32(0x0FFF) + ((u >> np.uint32(13)) & np.uint32(1))) & np.uint32(
        0xFFFFE000
    )
    return u.view(np.float32)


def _build_program():
    import concourse.tile as tile
    import concourse.mybir as mybir
    from concourse import bacc

    f32 = mybir.dt.float32
    f32r = mybir.dt.float32r

    nc = bacc.Bacc("TRN2", target_bir_lowering=False, debug=False, num_devices=8)

    xT = nc.dram_tensor("xT", [H, C], f32r, kind="ExternalInput").ap()
    w1T = nc.dram_tensor("w1T", [H, F], f32r, kind="ExternalInput").ap()
    w3T = nc.dram_tensor("w3T", [H, F], f32r, kind="ExternalInput").ap()
    w2T = nc.dram_tensor("w2T", [F, H], f32r, kind="ExternalInput").ap()
    sb = nc.dram_tensor("sb", [128, C], f32, kind="ExternalInput").ap()
    yT = nc.dram_tensor("yT", [H, C], f32, kind="ExternalOutput").ap()

    # DRAM views splitting the leading dim into (chunk, partition)
    xT_v = xT.rearrange("(k p) n -> p k n", p=128)    # [128, KCH, C]
    w1T_v = w1T.rearrange("(k p) f -> p k f", p=128)  # [128, KCH, F]
    w3T_v = w3T.rearrange("(k p) f -> p k f", p=128)
    w2T_v = w2T.rearrange("(k p) h -> p k h", p=128)  # [128, MCH, H]

    with tile.TileContext(nc) as tc:
        with tc.tile_pool(name="xp", bufs=1) as xp, \
             tc.tile_pool(name="hp", bufs=1) as hp, \
             tc.tile_pool(name="wp", bufs=3) as wp, \
             tc.tile_pool(name="sp", bufs=1) as sp, \
             tc.tile_pool(name="hs", bufs=3) as hsp, \
             tc.tile_pool(name="yp", bufs=3) as yp, \
             tc.tile_pool(name="ps", bufs=2, space="PSUM") as ps:

            st = None

            def load_w13(m):
                w1c = wp.tile([128, KCH * 128], f32r, tag="w1c", name="w1c")
                nc.sync.dma_start(
                    w1c[:].rearrange("p (k c) -> p k c", k=KCH),
                    w1T_v[:, :, m * 128:(m + 1) * 128],
                )
                w3c = wp.tile([128, KCH * 128], f32r, tag="w3c", name="w3c")
                nc.sync.dma_start(
                    w3c[:].rearrange("p (k c) -> p k c", k=KCH),
                    w3T_v[:, :, m * 128:(m + 1) * 128],
                )
                return w1c, w3c

            first = True
            for pstart, blocks in PHASES:
                plen = sum(blocks)

                # m=0 weight chunks first: they gate the first matmul and
                # are smaller than the token load
                w13_0 = load_w13(0) if first else None

                # ---- load this phase's tokens: one [128, nlen] tile per
                # (k-chunk, block) so the first block's matmuls start sooner
                xbs = []
                boff = 0
                for bi, nlen in enumerate(blocks):
                    xts = []
                    for k in range(KCH):
                        xt = xp.tile([128, 512], f32r, tag=f"x{k}b{bi}",
                                     name=f"x{k}b{bi}")
                        nc.sync.dma_start(
                            xt[:, :nlen],
                            xT_v[:, k, pstart + boff:pstart + boff + nlen],
                        )
                        xts.append(xt)
                    xbs.append(xts)
                    boff += nlen

                hts = [hp.tile([128, 1024], f32r, tag=f"h{m}", name=f"h{m}")
                       for m in range(MCH)]

                # ---- stage 1: h = silu(x@w1e.T) * (x@w3e.T), layout [F, tok]
                for m in range(MCH):
                    if m == 0 and w13_0 is not None:
                        w1c, w3c = w13_0
                        first = False
                    else:
                        w1c, w3c = load_w13(m)
                    noff = 0
                    for bi, nlen in enumerate(blocks):
                        p1 = ps.tile([128, 512], mybir.dt.float32, tag="p1")
                        p3 = ps.tile([128, 512], mybir.dt.float32, tag="p3")
                        for k in range(KCH):
                            nc.tensor.matmul(
                                p1[:, :nlen],
                                w1c[:, k * 128:(k + 1) * 128],
                                xbs[bi][k][:, :nlen],
                                start=(k == 0), stop=(k == KCH - 1),
                            )
                        for k in range(KCH):
                            nc.tensor.matmul(
                                p3[:, :nlen],
                                w3c[:, k * 128:(k + 1) * 128],
                                xbs[bi][k][:, :nlen],
                                start=(k == 0), stop=(k == KCH - 1),
                            )
                        hs = hsp.tile([128, 512], f32, tag="hs")
                        nc.scalar.activation(
                            hs[:, :nlen], p1[:, :nlen],
                            mybir.ActivationFunctionType.Silu,
                        )
                        nc.vector.tensor_mul(
                            hts[m][:, noff:noff + nlen], hs[:, :nlen], p3[:, :nlen]
                        )
                        noff += nlen

                # ---- stage 2: y = h.T @ w2e.T scaled by routing weight
                if st is None:
                    st = sp.tile([128, C], f32, tag="s", name="st")
                    nc.sync.dma_start(st[:], sb[:, :])
                for mh in range(H // 128):
                    w2c = wp.tile([128, MCH * 128], f32r, tag="w2c")
                    nc.sync.dma_start(
                        w2c[:].rearrange("p (k c) -> p k c", k=MCH),
                        w2T_v[:, :, mh * 128:(mh + 1) * 128],
                    )
                    noff = 0
                    for nlen in blocks:
                        py = ps.tile([128, 512], mybir.dt.float32, tag="py")
                        for k in range(MCH):
                            nc.tensor.matmul(
                                py[:, :nlen],
                                w2c[:, k * 128:(k + 1) * 128],
                                hts[k][:, noff:noff + nlen],
                                start=(k == 0), stop=(k == MCH - 1),
                            )
                        yt = yp.tile([128, 512], f32, tag="y")
                        nc.vector.tensor_mul(
                            yt[:, :nlen], py[:, :nlen],
                            st[:, pstart + noff:pstart + noff + nlen],
                        )
                        nc.sync.dma_start(
                            yT[mh * 128:(mh + 1) * 128,
                               pstart + noff:pstart + noff + nlen],
                            yt[:, :nlen],
                        )
                        noff += nlen

    nc.compile()
    return nc


def _get_program():
    if "nc" not in _COMPILED:
        _COMPILED["nc"] = _build_program()
    return _COMPILED["nc"]


def _host_routing(x: np.ndarray, gate_w: np.ndarray):
    """Replicate the reference router with jax-on-CPU (bit-identical).

    Falls back to numpy with the same semantics (stable softmax, top-2 with
    lower-index tie break) if the jax CPU backend is unavailable.
    """
    try:
        import jax
        import jax.numpy as jnp

        cpu = jax.local_devices(backend="cpu")[0]
        with jax.default_device(cpu):
            xj = jnp.asarray(x)
            gj = jnp.asarray(gate_w)
            router_logits = xj @ gj.T
            probs = jax.nn.softmax(router_logits.astype(jnp.float32), axis=-1)
            rw, selected = jax.lax.top_k(probs, TOPK)
            rw = rw / jnp.sum(rw, axis=-1, keepdims=True)
            return (
                np.asarray(router_logits, dtype=np.float32),
                np.asarray(rw, dtype=np.float32),
                np.asarray(selected),
            )
    except Exception:
        router_logits = (x @ gate_w.T).astype(np.float32)
        z = router_logits - router_logits.max(axis=-1, keepdims=True)
        ez = np.exp(z, dtype=np.float32)
        probs = ez / ez.sum(axis=-1, keepdims=True)
        i1 = probs.argmax(axis=-1)
        p2 = probs.copy()
        p2[np.arange(T), i1] = -np.inf
        i2 = p2.argmax(axis=-1)
        selected = np.stack([i1, i2], axis=1)
        rw = np.take_along_axis(probs, selected, axis=1)
        rw = (rw / rw.sum(axis=-1, keepdims=True)).astype(np.float32)
        return router_logits, rw, selected


def _silu32(u: np.ndarray) -> np.ndarray:
    return (u / (1.0 + np.exp(-u))).astype(np.float32)


def kernel(hidden_states, gate_w, w1, w3, w2):
    import time as _time

    from concourse.bass_utils import run_bass_kernel_spmd

    _tlog = []
    _t0 = _time.time()

    hidden_states = np.asarray(hidden_states, dtype=np.float32)
    gate_w = np.asarray(gate_w, dtype=np.float32)
    w1 = np.asarray(w1, dtype=np.float32)
    w3 = np.asarray(w3, dtype=np.float32)
    w2 = np.asarray(w2, dtype=np.float32)

    x = hidden_states.reshape(T, H)
    router_logits, rw, selected = _host_routing(x, gate_w)
    _tlog.append(("routing", _time.time() - _t0)); _t0 = _time.time()

    # per-expert token lists + weights
    in_maps = []
    dev_idx = []
    host_work = []  # (e, idx, weights)
    for e in range(E):
        mask = selected == e  # [T, K]
        tok = np.nonzero(mask.any(axis=1))[0]
        wgt = (rw * mask).sum(axis=1)[tok].astype(np.float32)
        if len(tok) > C:
            host_work.append((e, tok[C:], wgt[C:]))
            tok, wgt = tok[:C], wgt[:C]
        dev_idx.append(tok)

        n = len(tok)
        xg = np.zeros((C, H), dtype=np.float32)
        xg[:n] = x[tok]
        sv = np.zeros((C,), dtype=np.float32)
        sv[:n] = wgt
        in_maps.append({
            "xT": _round_tf32(xg.T),
            "w1T": _round_tf32(w1[e].T),
            "w3T": _round_tf32(w3[e].T),
            "w2T": _round_tf32(w2[e].T),
            "sb": np.ascontiguousarray(
                np.broadcast_to(sv[None, :], (128, C))
            ),
        })

    _tlog.append(("prep", _time.time() - _t0)); _t0 = _time.time()

    nc = _get_program()
    _tlog.append(("build", _time.time() - _t0)); _t0 = _time.time()
    trace = bool(int(os.environ.get("MOE_KERNEL_TRACE", "0")))
    res = run_bass_kernel_spmd(nc, in_maps, list(range(8)), trace=trace)
    if trace:
        kernel.last_exec_time_ns = res.exec_time_ns
        kernel.last_results = res
    _tlog.append(("device", _time.time() - _t0)); _t0 = _time.time()

    out_T = np.zeros((H, T), dtype=np.float32)
    for e in range(E):
        tok = dev_idx[e]
        out_T[:, tok] += res.results[e]["yT"][:, :len(tok)]

    for e, tok, wgt in host_work:
        xo = x[tok]
        g = _silu32(xo @ w1[e].T) * (xo @ w3[e].T)
        out_T[:, tok] += (wgt[:, None] * (g @ w2[e].T)).T

    out = np.ascontiguousarray(out_T.T).reshape(B, S, H)
    _tlog.append(("combine", _time.time() - _t0))
    if os.environ.get("MOE_KERNEL_TIME"):
        print("kernel wall:", " ".join(f"{k}={v:.2f}s" for k, v in _tlog))
    return out, router_logits


# revision 22
# speedup vs baseline: 1.1960x; 1.1960x over previous
"""Mixtral sparse MoE block on 8 Trainium2 NeuronCores (expert parallelism).

Contract: kernel(**inputs) takes FULL unsharded numpy inputs and returns the
FULL output, matching reference.reference() (a tuple: (out[B,S,H], router_logits[T,E])).

Strategy:
  - Router (logits, softmax, top-2, weight normalization) runs on host with
    jax-on-CPU, replicating the reference ops so expert selection is
    bit-identical to the reference.
  - Tokens are dispatched host-side to their selected experts; core e runs
    expert e's FFN (silu(x@w1.T)*(x@w3.T))@w2.T over its (padded) token set
    at fp32r (TF32) precision on the PE, scaled by the routing weight.
  - Host scatters the per-expert outputs back and sums (each token has
    exactly 2 contributions). Any tokens beyond the per-expert capacity are
    computed on host in fp32 (never triggers for balanced routing).
"""

import os
import sys
import types

import numpy as np

for _p in ("/opt/trn_rl_repo",):
    if _p not in sys.path and os.path.isdir(_p):
        sys.path.append(_p)

B, S, H, F, E, TOPK = 4, 2048, 1024, 2048, 8, 2
T = B * S
C = 2048  # per-expert token capacity (multiple of 512)
KCH = H // 128   # 8  contraction chunks for stage 1
MCH = F // 128   # 16 F chunks
PHASES = ((0, (512, 512)), (1024, (512, 512)))  # (start, block lens)

_COMPILED = {}


def _round_tf32(a: np.ndarray) -> np.ndarray:
    """Round fp32 -> tf32 (10-bit mantissa), RNE, as fp32 storage."""
    u = np.ascontiguousarray(a, dtype=np.float32).view(np.uint32)
    u = (u + np.uint32(0x0FFF) + ((u >> np.uint32(13)) & np.uint32(1))) & np.uint32(
        0xFFFFE000
    )
    return u.view(np.float32)


def _build_program():
    import concourse.tile as tile
    import concourse.mybir as mybir
    from concourse import bacc

    f32 = mybir.dt.float32
    f32r = mybir.dt.float32r

    nc = bacc.Bacc("TRN2", target_bir_lowering=False, debug=False, num_devices=8)

    xT = nc.dram_tensor("xT", [H, C], f32r, kind="ExternalInput").ap()
    w1T = nc.dram_tensor("w1T", [H, F], f32r, kind="ExternalInput").ap()
    w3T = nc.dram_tensor("w3T", [H, F], f32r, kind="ExternalInput").ap()
    w2T = nc.dram_tensor("w2T", [F, H], f32r, kind="ExternalInput").ap()
    sb = nc.dram_tensor("sb", [128, C], f32, kind="ExternalInput").ap()
    yT = nc.dram_tensor("yT", [H, C], f32, kind="ExternalOutput").ap()

    # DRAM views splitting the leading dim into (chunk, partition)
    xT_v = xT.rearrange("(k p) n -> p k n", p=128)    # [128, KCH, C]
    w1T_v = w1T.rearrange("(k p) f -> p k f", p=128)  # [128, KCH, F]
    w3T_v = w3T.rearrange("(k p) f -> p k f", p=128)
    w2T_v = w2T.rearrange("(k p) h -> p k h", p=128)  # [128, MCH, H]

    with tile.TileContext(nc) as tc:
        with tc.tile_pool(name="xp", bufs=1) as xp, \
             tc.tile_pool(name="hp", bufs=1) as hp, \
             tc.tile_pool(name="wp", bufs=3) as wp, \
             tc.tile_pool(name="sp", bufs=1) as sp, \
             tc.tile_pool(name="hs", bufs=3) as hsp, \
             tc.tile_pool(name="yp", bufs=3) as yp, \
             tc.tile_pool(name="ps", bufs=2, space="PSUM") as ps:

            st = None

            def load_w13(m):
                w1c = wp.tile([128, KCH * 128], f32r, tag="w1c", name="w1c")
                nc.sync.dma_start(
                    w1c[:].rearrange("p (k c) -> p k c", k=KCH),
                    w1T_v[:, :, m * 128:(m + 1) * 128],
                )
                w3c = wp.tile([128, KCH * 128], f32r, tag="w3c", name="w3c")
                nc.sync.dma_start(
                    w3c[:].rearrange("p (k c) -> p k c", k=KCH),
                    w3T_v[:, :, m * 128:(m + 1) * 128],
                )
                return w1c, w3c

            first = True
            for pstart, blocks in PHASES:
                plen = sum(blocks)

                # m=0 weight chunks first: they gate the first matmul and
                # are smaller than the token load
                w13_0 = load_w13(0) if first else None

                # ---- load this phase's tokens: 8 chunks of [128, plen]
                xts = []
                for k in range(KCH):
                    xt = xp.tile([128, 1024], f32r, tag=f"x{k}", name=f"x{k}")
                    nc.sync.dma_start(
                        xt[:, :plen], xT_v[:, k, pstart:pstart + plen]
                    )
                    xts.append(xt)

                hts = [hp.tile([128, 1024], f32r, tag=f"h{m}", name=f"h{m}")
                       for m in range(MCH)]

                # ---- stage 1: h = silu(x@w1e.T) * (x@w3e.T), layout [F, tok]
                for m in range(MCH):
                    if m == 0 and w13_0 is not None:
                        w1c, w3c = w13_0
                        first = False
                    else:
                        w1c, w3c = load_w13(m)
                    noff = 0
                    for nlen in blocks:
                        p1 = ps.tile([128, 512], mybir.dt.float32, tag="p1")
                        p3 = ps.tile([128, 512], mybir.dt.float32, tag="p3")
                        for k in range(KCH):
                            nc.tensor.matmul(
                                p1[:, :nlen],
                                w1c[:, k * 128:(k + 1) * 128],
                                xts[k][:, noff:noff + nlen],
                                start=(k == 0), stop=(k == KCH - 1),
                            )
                        for k in range(KCH):
                            nc.tensor.matmul(
                                p3[:, :nlen],
                                w3c[:, k * 128:(k + 1) * 128],
                                xts[k][:, noff:noff + nlen],
                                start=(k == 0), stop=(k == KCH - 1),
                            )
                        hs = hsp.tile([128, 512], f32, tag="hs")
                        nc.scalar.activation(
                            hs[:, :nlen], p1[:, :nlen],
                            mybir.ActivationFunctionType.Silu,
                        )
                        nc.vector.tensor_mul(
                            hts[m][:, noff:noff + nlen], hs[:, :nlen], p3[:, :nlen]
                        )
                        noff += nlen

                # ---- stage 2: y = h.T @ w2e.T scaled by routing weight
                if st is None:
                    st = sp.tile([128, C], f32, tag="s", name="st")
                    nc.sync.dma_start(st[:], sb[:, :])
                for mh in range(H // 128):
                    w2c = wp.tile([128, MCH * 128], f32r, tag="w2c")
                    nc.sync.dma_start(
                        w2c[:].rearrange("p (k c) -> p k c", k=MCH),
                        w2T_v[:, :, mh * 128:(mh + 1) * 128],
                    )
                    noff = 0
                    for nlen in blocks:
                        py = ps.tile([128, 512], mybir.dt.float32, tag="py")
                        for k in range(MCH):
                            nc.tensor.matmul(
                                py[:, :nlen],
                                w2c[:, k * 128:(k + 1) * 128],
                                hts[k][:, noff:noff + nlen],
                                start=(k == 0), stop=(k == MCH - 1),
                            )
                        yt = yp.tile([128, 512], f32, tag="y")
                        nc.vector.tensor_mul(
                            yt[:, :nlen], py[:, :nlen],
                            st[:, pstart + noff:pstart + noff + nlen],
                        )
                        nc.sync.dma_start(
                            yT[mh * 128:(mh + 1) * 128,
                               pstart + noff:pstart + noff + nlen],
                            yt[:, :nlen],
                        )
                        noff += nlen

    nc.compile()
    return nc


def _get_program():
    if "nc" not in _COMPILED:
        _COMPILED["nc"] = _build_program()
    return _COMPILED["nc"]


def _host_routing(x: np.ndarray, gate_w: np.ndarray):
    """Replicate the reference router with jax-on-CPU (bit-identical).

    Falls back to numpy with the same semantics (stable softmax, top-2 with
    lower-index tie break) if the jax CPU backend is unavailable.
    """
    try:
        import jax
        import jax.numpy as jnp

        cpu = jax.local_devices(backend="cpu")[0]
        with jax.default_device(cpu):
            xj = jnp.asarray(x)
            gj = jnp.asarray(gate_w)
            router_logits = xj @ gj.T
            probs = jax.nn.softmax(router_logits.astype(jnp.float32), axis=-1)
            rw, selected = jax.lax.top_k(probs, TOPK)
            rw = rw / jnp.sum(rw, axis=-1, keepdims=True)
            return (
                np.asarray(router_logits, dtype=np.float32),
                np.asarray(rw, dtype=np.float32),
                np.asarray(selected),
            )
    except Exception:
        router_logits = (x @ gate_w.T).astype(np.float32)
        z = router_logits - router_logits.max(axis=-1, keepdims=True)
        ez = np.exp(z, dtype=np.float32)
        probs = ez / ez.sum(axis=-1, keepdims=True)
        i1 = probs.argmax(axis=-1)
        p2 = probs.copy()
        p2[np.arange(T), i1] = -np.inf
        i2 = p2.argmax(axis=-1)
        selected = np.stack([i1, i2], axis=1)
        rw = np.take_along_axis(probs, selected, axis=1)
        rw = (rw / rw.sum(axis=-1, keepdims=True)).astype(np.float32)
        return router_logits, rw, selected


def _silu32(u: np.ndarray) -> np.ndarray:
    return (u / (1.0 + np.exp(-u))).astype(np.float32)


def kernel(hidden_states, gate_w, w1, w3, w2):
    import time as _time

    from concourse.bass_utils import run_bass_kernel_spmd

    _tlog = []
    _t0 = _time.time()

    hidden_states = np.asarray(hidden_states, dtype=np.float32)
    gate_w = np.asarray(gate_w, dtype=np.float32)
    w1 = np.asarray(w1, dtype=np.float32)
    w3 = np.asarray(w3, dtype=np.float32)
    w2 = np.asarray(w2, dtype=np.float32)

    x = hidden_states.reshape(T, H)
    router_logits, rw, selected = _host_routing(x, gate_w)
    _tlog.append(("routing", _time.time() - _t0)); _t0 = _time.time()

    # per-expert token lists + weights
    in_maps = []
    dev_idx = []
    host_work = []  # (e, idx, weights)
    for e in range(E):
        mask = selected == e  # [T, K]
        tok = np.nonzero(mask.any(axis=1))[0]
        wgt = (rw * mask).sum(axis=1)[tok].astype(np.float32)
        if len(tok) > C:
            host_work.append((e, tok[C:], wgt[C:]))
            tok, wgt = tok[:C], wgt[:C]
        dev_idx.append(tok)

        n = len(tok)
        xg = np.zeros((C, H), dtype=np.float32)
        xg[:n] = x[tok]
        sv = np.zeros((C,), dtype=np.float32)
        sv[:n] = wgt
        in_maps.append({
            "xT": _round_tf32(xg.T),
            "w1T": _round_tf32(w1[e].T),
            "w3T": _round_tf32(w3[e].T),
            "w2T": _round_tf32(w2[e].T),
            "sb": np.ascontiguousarray(
                np.broadcast_to(sv[None, :], (128, C))
            ),
        })

    _tlog.append(("prep", _time.time() - _t0)); _t0 = _time.time()

    nc = _get_program()
    _tlog.append(("build", _time.time() - _t0)); _t0 = _time.time()
    trace = bool(int(os.environ.get("MOE_KERNEL_TRACE", "0")))
    res = run_bass_kernel_spmd(nc, in_maps, list(range(8)), trace=trace)
    if trace:
        kernel.last_exec_time_ns = res.exec_time_ns
        kernel.last_results = res
    _tlog.append(("device", _time.time() - _t0)); _t0 = _time.time()

    out_T = np.zeros((H, T), dtype=np.float32)
    for e in range(E):
        tok = dev_idx[e]
        out_T[:, tok] += res.results[e]["yT"][:, :len(tok)]

    for e, tok, wgt in host_work:
        xo = x[tok]
        g = _silu32(xo @ w1[e].T) * (xo @ w3[e].T)
        out_T[:, tok] += (wgt[:, None] * (g @ w2[e].T)).T

    out = np.ascontiguousarray(out_T.T).reshape(B, S, H)
    _tlog.append(("combine", _time.time() - _t0))
    if os.environ.get("MOE_KERNEL_TIME"):
        print("kernel wall:", " ".join(f"{k}={v:.2f}s" for k, v in _tlog))
    return out, router_logits


# revision 23
# speedup vs baseline: 1.2581x; 1.0519x over previous
"""Mixtral sparse MoE block on 8 Trainium2 NeuronCores (expert parallelism).

Contract: kernel(**inputs) takes FULL unsharded numpy inputs and returns the
FULL output, matching reference.reference() (a tuple: (out[B,S,H], router_logits[T,E])).

Strategy:
  - Router (logits, softmax, top-2, weight normalization) runs on host with
    jax-on-CPU, replicating the reference ops so expert selection is
    bit-identical to the reference.
  - Tokens are dispatched host-side to their selected experts; core e runs
    expert e's FFN (silu(x@w1.T)*(x@w3.T))@w2.T over its (padded) token set
    at fp32r (TF32) precision on the PE, scaled by the routing weight.
  - Host scatters the per-expert outputs back and sums (each token has
    exactly 2 contributions). Any tokens beyond the per-expert capacity are
    computed on host in fp32 (never triggers for balanced routing).
"""

import os
import sys
import types

import numpy as np

for _p in ("/opt/trn_rl_repo",):
    if _p not in sys.path and os.path.isdir(_p):
        sys.path.append(_p)

B, S, H, F, E, TOPK = 4, 2048, 1024, 2048, 8, 2
T = B * S
C = 2048  # per-expert token capacity (multiple of 512)
KCH = H // 128   # 8  contraction chunks for stage 1
MCH = F // 128   # 16 F chunks
PHASES = ((0, (512, 512, 512, 512)),)  # single phase (fp16 fits resident)

_COMPILED = {}


def _round_tf32(a: np.ndarray) -> np.ndarray:
    """Round fp32 -> tf32 (10-bit mantissa), RNE, as fp32 storage."""
    u = np.ascontiguousarray(a, dtype=np.float32).view(np.uint32)
    u = (u + np.uint32(0x0FFF) + ((u >> np.uint32(13)) & np.uint32(1))) & np.uint32(
        0xFFFFE000
    )
    return u.view(np.float32)


def _build_program():
    import concourse.tile as tile
    import concourse.mybir as mybir
    from concourse import bacc

    f32 = mybir.dt.float32
    f16 = mybir.dt.float16

    nc = bacc.Bacc("TRN2", target_bir_lowering=False, debug=False, num_devices=8)

    xT = nc.dram_tensor("xT", [H, C], f16, kind="ExternalInput").ap()
    w1T = nc.dram_tensor("w1T", [H, F], f16, kind="ExternalInput").ap()
    w3T = nc.dram_tensor("w3T", [H, F], f16, kind="ExternalInput").ap()
    w2T = nc.dram_tensor("w2T", [F, H], f16, kind="ExternalInput").ap()
    sb = nc.dram_tensor("sb", [128, C], f32, kind="ExternalInput").ap()
    yT = nc.dram_tensor("yT", [H, C], f32, kind="ExternalOutput").ap()

    # DRAM views splitting the leading dim into (chunk, partition)
    xT_v = xT.rearrange("(k p) n -> p k n", p=128)    # [128, KCH, C]
    w1T_v = w1T.rearrange("(k p) f -> p k f", p=128)  # [128, KCH, F]
    w3T_v = w3T.rearrange("(k p) f -> p k f", p=128)
    w2T_v = w2T.rearrange("(k p) h -> p k h", p=128)  # [128, MCH, H]

    with tile.TileContext(nc) as tc:
        with tc.tile_pool(name="xp", bufs=1) as xp, \
             tc.tile_pool(name="hp", bufs=1) as hp, \
             tc.tile_pool(name="wp", bufs=3) as wp, \
             tc.tile_pool(name="sp", bufs=1) as sp, \
             tc.tile_pool(name="hs", bufs=3) as hsp, \
             tc.tile_pool(name="yp", bufs=3) as yp, \
             tc.tile_pool(name="ps", bufs=2, space="PSUM") as ps:

            st = None

            def load_w13(m):
                w1c = wp.tile([128, KCH * 128], f16, tag="w1c", name="w1c")
                nc.sync.dma_start(
                    w1c[:].rearrange("p (k c) -> p k c", k=KCH),
                    w1T_v[:, :, m * 128:(m + 1) * 128],
                )
                w3c = wp.tile([128, KCH * 128], f16, tag="w3c", name="w3c")
                nc.sync.dma_start(
                    w3c[:].rearrange("p (k c) -> p k c", k=KCH),
                    w3T_v[:, :, m * 128:(m + 1) * 128],
                )
                return w1c, w3c

            first = True
            for pstart, blocks in PHASES:
                plen = sum(blocks)

                # m=0 weight chunks first: they gate the first matmul and
                # are smaller than the token load
                w13_0 = load_w13(0) if first else None

                # ---- load this phase's tokens: 8 chunks of [128, plen]
                xts = []
                for k in range(KCH):
                    xt = xp.tile([128, 2048], f16, tag=f"x{k}", name=f"x{k}")
                    nc.sync.dma_start(
                        xt[:, :plen], xT_v[:, k, pstart:pstart + plen]
                    )
                    xts.append(xt)

                hts = [hp.tile([128, 2048], f16, tag=f"h{m}", name=f"h{m}")
                       for m in range(MCH)]

                # ---- stage 1: h = silu(x@w1e.T) * (x@w3e.T), layout [F, tok]
                for m in range(MCH):
                    if m == 0 and w13_0 is not None:
                        w1c, w3c = w13_0
                        first = False
                    else:
                        w1c, w3c = load_w13(m)
                    noff = 0
                    for nlen in blocks:
                        p1 = ps.tile([128, 512], mybir.dt.float32, tag="p1")
                        p3 = ps.tile([128, 512], mybir.dt.float32, tag="p3")
                        for k in range(KCH):
                            nc.tensor.matmul(
                                p1[:, :nlen],
                                w1c[:, k * 128:(k + 1) * 128],
                                xts[k][:, noff:noff + nlen],
                                start=(k == 0), stop=(k == KCH - 1),
                            )
                        for k in range(KCH):
                            nc.tensor.matmul(
                                p3[:, :nlen],
                                w3c[:, k * 128:(k + 1) * 128],
                                xts[k][:, noff:noff + nlen],
                                start=(k == 0), stop=(k == KCH - 1),
                            )
                        hs = hsp.tile([128, 512], f32, tag="hs")
                        nc.scalar.activation(
                            hs[:, :nlen], p1[:, :nlen],
                            mybir.ActivationFunctionType.Silu,
                        )
                        nc.vector.tensor_mul(
                            hts[m][:, noff:noff + nlen], hs[:, :nlen], p3[:, :nlen]
                        )
                        noff += nlen

                # ---- stage 2: y = h.T @ w2e.T scaled by routing weight
                if st is None:
                    st = sp.tile([128, C], f32, tag="s", name="st")
                    nc.sync.dma_start(st[:], sb[:, :])
                for mh in range(H // 128):
                    w2c = wp.tile([128, MCH * 128], f16, tag="w2c")
                    nc.sync.dma_start(
                        w2c[:].rearrange("p (k c) -> p k c", k=MCH),
                        w2T_v[:, :, mh * 128:(mh + 1) * 128],
                    )
                    noff = 0
                    for nlen in blocks:
                        py = ps.tile([128, 512], mybir.dt.float32, tag="py")
                        for k in range(MCH):
                            nc.tensor.matmul(
                                py[:, :nlen],
                                w2c[:, k * 128:(k + 1) * 128],
                                hts[k][:, noff:noff + nlen],
                                start=(k == 0), stop=(k == MCH - 1),
                            )
                        yt = yp.tile([128, 512], f32, tag="y")
                        nc.vector.tensor_mul(
                            yt[:, :nlen], py[:, :nlen],
                            st[:, pstart + noff:pstart + noff + nlen],
                        )
                        nc.sync.dma_start(
                            yT[mh * 128:(mh + 1) * 128,
                               pstart + noff:pstart + noff + nlen],
                            yt[:, :nlen],
                        )
                        noff += nlen

    nc.compile()
    return nc


def _get_program():
    if "nc" not in _COMPILED:
        _COMPILED["nc"] = _build_program()
    return _COMPILED["nc"]


def _host_routing(x: np.ndarray, gate_w: np.ndarray):
    """Replicate the reference router with jax-on-CPU (bit-identical).

    Falls back to numpy with the same semantics (stable softmax, top-2 with
    lower-index tie break) if the jax CPU backend is unavailable.
    """
    try:
        import jax
        import jax.numpy as jnp

        cpu = jax.local_devices(backend="cpu")[0]
        with jax.default_device(cpu):
            xj = jnp.asarray(x)
            gj = jnp.asarray(gate_w)
            router_logits = xj @ gj.T
            probs = jax.nn.softmax(router_logits.astype(jnp.float32), axis=-1)
            rw, selected = jax.lax.top_k(probs, TOPK)
            rw = rw / jnp.sum(rw, axis=-1, keepdims=True)
            return (
                np.asarray(router_logits, dtype=np.float32),
                np.asarray(rw, dtype=np.float32),
                np.asarray(selected),
            )
    except Exception:
        router_logits = (x @ gate_w.T).astype(np.float32)
        z = router_logits - router_logits.max(axis=-1, keepdims=True)
        ez = np.exp(z, dtype=np.float32)
        probs = ez / ez.sum(axis=-1, keepdims=True)
        i1 = probs.argmax(axis=-1)
        p2 = probs.copy()
        p2[np.arange(T), i1] = -np.inf
        i2 = p2.argmax(axis=-1)
        selected = np.stack([i1, i2], axis=1)
        rw = np.take_along_axis(probs, selected, axis=1)
        rw = (rw / rw.sum(axis=-1, keepdims=True)).astype(np.float32)
        return router_logits, rw, selected


def _silu32(u: np.ndarray) -> np.ndarray:
    return (u / (1.0 + np.exp(-u))).astype(np.float32)


def kernel(hidden_states, gate_w, w1, w3, w2):
    import time as _time

    from concourse.bass_utils import run_bass_kernel_spmd

    _tlog = []
    _t0 = _time.time()

    hidden_states = np.asarray(hidden_states, dtype=np.float32)
    gate_w = np.asarray(gate_w, dtype=np.float32)
    w1 = np.asarray(w1, dtype=np.float32)
    w3 = np.asarray(w3, dtype=np.float32)
    w2 = np.asarray(w2, dtype=np.float32)

    x = hidden_states.reshape(T, H)
    router_logits, rw, selected = _host_routing(x, gate_w)
    _tlog.append(("routing", _time.time() - _t0)); _t0 = _time.time()

    # per-expert token lists + weights
    in_maps = []
    dev_idx = []
    host_work = []  # (e, idx, weights)
    for e in range(E):
        mask = selected == e  # [T, K]
        tok = np.nonzero(mask.any(axis=1))[0]
        wgt = (rw * mask).sum(axis=1)[tok].astype(np.float32)
        if len(tok) > C:
            host_work.append((e, tok[C:], wgt[C:]))
            tok, wgt = tok[:C], wgt[:C]
        dev_idx.append(tok)

        n = len(tok)
        xg = np.zeros((C, H), dtype=np.float32)
        xg[:n] = x[tok]
        sv = np.zeros((C,), dtype=np.float32)
        sv[:n] = wgt
        in_maps.append({
            "xT": np.ascontiguousarray(xg.T, dtype=np.float16),
            "w1T": np.ascontiguousarray(w1[e].T, dtype=np.float16),
            "w3T": np.ascontiguousarray(w3[e].T, dtype=np.float16),
            "w2T": np.ascontiguousarray(w2[e].T, dtype=np.float16),
            "sb": np.ascontiguousarray(
                np.broadcast_to(sv[None, :], (128, C))
            ),
        })

    _tlog.append(("prep", _time.time() - _t0)); _t0 = _time.time()

    nc = _get_program()
    _tlog.append(("build", _time.time() - _t0)); _t0 = _time.time()
    trace = bool(int(os.environ.get("MOE_KERNEL_TRACE", "0")))
    res = run_bass_kernel_spmd(nc, in_maps, list(range(8)), trace=trace)
    if trace:
        kernel.last_exec_time_ns = res.exec_time_ns
        kernel.last_results = res
    _tlog.append(("device", _time.time() - _t0)); _t0 = _time.time()

    out_T = np.zeros((H, T), dtype=np.float32)
    for e in range(E):
        tok = dev_idx[e]
        out_T[:, tok] += res.results[e]["yT"][:, :len(tok)]

    for e, tok, wgt in host_work:
        xo = x[tok]
        g = _silu32(xo @ w1[e].T) * (xo @ w3[e].T)
        out_T[:, tok] += (wgt[:, None] * (g @ w2[e].T)).T

    out = np.ascontiguousarray(out_T.T).reshape(B, S, H)
    _tlog.append(("combine", _time.time() - _t0))
    if os.environ.get("MOE_KERNEL_TIME"):
        print("kernel wall:", " ".join(f"{k}={v:.2f}s" for k, v in _tlog))
    return out, router_logits


# revision 24
# speedup vs baseline: 1.2596x; 1.0011x over previous
"""Mixtral sparse MoE block on 8 Trainium2 NeuronCores (expert parallelism).

Contract: kernel(**inputs) takes FULL unsharded numpy inputs and returns the
FULL output, matching reference.reference() (a tuple: (out[B,S,H], router_logits[T,E])).

Strategy:
  - Router (logits, softmax, top-2, weight normalization) runs on host with
    jax-on-CPU, replicating the reference ops so expert selection is
    bit-identical to the reference.
  - Tokens are dispatched host-side to their selected experts; core e runs
    expert e's FFN (silu(x@w1.T)*(x@w3.T))@w2.T over its (padded) token set
    with fp16 operands (10-bit mantissa, same precision class as TF32 for
    these well-scaled values) and fp32 PSUM accumulation, scaled by the
    routing weight. fp16 streams at the PE's ideal 1 cyc/row and halves
    HBM traffic vs fp32r.
  - Host scatters the per-expert outputs back and sums (each token has
    exactly 2 contributions). Any tokens beyond the per-expert capacity are
    computed on host in fp32 (never triggers for balanced routing).
"""

import os
import sys
import types

import numpy as np

for _p in ("/opt/trn_rl_repo",):
    if _p not in sys.path and os.path.isdir(_p):
        sys.path.append(_p)

B, S, H, F, E, TOPK = 4, 2048, 1024, 2048, 8, 2
T = B * S
C = 2048  # per-expert token capacity (multiple of 512)
KCH = H // 128   # 8  contraction chunks for stage 1
MCH = F // 128   # 16 F chunks
PHASES = ((0, (512, 512, 512, 512)),)  # single phase (fp16 fits resident)

_COMPILED = {}


def _build_program():
    import concourse.tile as tile
    import concourse.mybir as mybir
    from concourse import bacc

    f32 = mybir.dt.float32
    f16 = mybir.dt.float16

    nc = bacc.Bacc("TRN2", target_bir_lowering=False, debug=False, num_devices=8)

    xT = nc.dram_tensor("xT", [H, C], f16, kind="ExternalInput").ap()
    w1T = nc.dram_tensor("w1T", [H, F], f16, kind="ExternalInput").ap()
    w3T = nc.dram_tensor("w3T", [H, F], f16, kind="ExternalInput").ap()
    w2T = nc.dram_tensor("w2T", [F, H], f16, kind="ExternalInput").ap()
    sb = nc.dram_tensor("sb", [128, C], f32, kind="ExternalInput").ap()
    yT = nc.dram_tensor("yT", [H, C], f32, kind="ExternalOutput").ap()

    # DRAM views splitting the leading dim into (chunk, partition)
    xT_v = xT.rearrange("(k p) n -> p k n", p=128)    # [128, KCH, C]
    w1T_v = w1T.rearrange("(k p) f -> p k f", p=128)  # [128, KCH, F]
    w3T_v = w3T.rearrange("(k p) f -> p k f", p=128)
    w2T_v = w2T.rearrange("(k p) h -> p k h", p=128)  # [128, MCH, H]

    with tile.TileContext(nc) as tc:
        with tc.tile_pool(name="xp", bufs=1) as xp, \
             tc.tile_pool(name="hp", bufs=1) as hp, \
             tc.tile_pool(name="wp", bufs=3) as wp, \
             tc.tile_pool(name="sp", bufs=1) as sp, \
             tc.tile_pool(name="hs", bufs=3) as hsp, \
             tc.tile_pool(name="yp", bufs=3) as yp, \
             tc.tile_pool(name="ps", bufs=2, space="PSUM") as ps:

            st = None

            def load_w13(m):
                w1c = wp.tile([128, KCH * 128], f16, tag="w1c", name="w1c")
                nc.sync.dma_start(
                    w1c[:].rearrange("p (k c) -> p k c", k=KCH),
                    w1T_v[:, :, m * 128:(m + 1) * 128],
                )
                w3c = wp.tile([128, KCH * 128], f16, tag="w3c", name="w3c")
                nc.sync.dma_start(
                    w3c[:].rearrange("p (k c) -> p k c", k=KCH),
                    w3T_v[:, :, m * 128:(m + 1) * 128],
                )
                return w1c, w3c

            first = True
            for pstart, blocks in PHASES:
                plen = sum(blocks)

                # m=0 weight chunks first: they gate the first matmul and
                # are smaller than the token load
                w13_0 = load_w13(0) if first else None

                # ---- load this phase's tokens: 8 chunks of [128, plen]
                xts = []
                for k in range(KCH):
                    xt = xp.tile([128, 2048], f16, tag=f"x{k}", name=f"x{k}")
                    nc.sync.dma_start(
                        xt[:, :plen], xT_v[:, k, pstart:pstart + plen]
                    )
                    xts.append(xt)

                hts = [hp.tile([128, 2048], f16, tag=f"h{m}", name=f"h{m}")
                       for m in range(MCH)]

                # ---- stage 1: h = silu(x@w1e.T) * (x@w3e.T), layout [F, tok]
                for m in range(MCH):
                    if m == 0 and w13_0 is not None:
                        w1c, w3c = w13_0
                        first = False
                    else:
                        w1c, w3c = load_w13(m)
                    noff = 0
                    for nlen in blocks:
                        p1 = ps.tile([128, 512], mybir.dt.float32, tag="p1")
                        p3 = ps.tile([128, 512], mybir.dt.float32, tag="p3")
                        for k in range(KCH):
                            nc.tensor.matmul(
                                p1[:, :nlen],
                                w1c[:, k * 128:(k + 1) * 128],
                                xts[k][:, noff:noff + nlen],
                                start=(k == 0), stop=(k == KCH - 1),
                            )
                        for k in range(KCH):
                            nc.tensor.matmul(
                                p3[:, :nlen],
                                w3c[:, k * 128:(k + 1) * 128],
                                xts[k][:, noff:noff + nlen],
                                start=(k == 0), stop=(k == KCH - 1),
                            )
                        hs = hsp.tile([128, 512], f32, tag="hs")
                        nc.scalar.activation(
                            hs[:, :nlen], p1[:, :nlen],
                            mybir.ActivationFunctionType.Silu,
                        )
                        nc.vector.tensor_mul(
                            hts[m][:, noff:noff + nlen], hs[:, :nlen], p3[:, :nlen]
                        )
                        noff += nlen

                # ---- stage 2: y = h.T @ w2e.T scaled by routing weight
                if st is None:
                    st = sp.tile([128, C], f32, tag="s", name="st")
                    nc.sync.dma_start(st[:], sb[:, :])
                for mh in range(H // 128):
                    w2c = wp.tile([128, MCH * 128], f16, tag="w2c")
                    nc.sync.dma_start(
                        w2c[:].rearrange("p (k c) -> p k c", k=MCH),
                        w2T_v[:, :, mh * 128:(mh + 1) * 128],
                    )
                    noff = 0
                    for nlen in blocks:
                        py = ps.tile([128, 512], mybir.dt.float32, tag="py")
                        for k in range(MCH):
                            nc.tensor.matmul(
                                py[:, :nlen],
                                w2c[:, k * 128:(k + 1) * 128],
                                hts[k][:, noff:noff + nlen],
                                start=(k == 0), stop=(k == MCH - 1),
                            )
                        yt = yp.tile([128, 512], f32, tag="y")
                        nc.vector.tensor_mul(
                            yt[:, :nlen], py[:, :nlen],
                            st[:, pstart + noff:pstart + noff + nlen],
                        )
                        nc.sync.dma_start(
                            yT[mh * 128:(mh + 1) * 128,
                               pstart + noff:pstart + noff + nlen],
                            yt[:, :nlen],
                        )
                        noff += nlen

    nc.compile()
    return nc


def _get_program():
    if "nc" not in _COMPILED:
        _COMPILED["nc"] = _build_program()
    return _COMPILED["nc"]


def _host_routing(x: np.ndarray, gate_w: np.ndarray):
    """Replicate the reference router with jax-on-CPU (bit-identical).

    Falls back to numpy with the same semantics (stable softmax, top-2 with
    lower-index tie break) if the jax CPU backend is unavailable.
    """
    try:
        import jax
        import jax.numpy as jnp

        cpu = jax.local_devices(backend="cpu")[0]
        with jax.default_device(cpu):
            xj = jnp.asarray(x)
            gj = jnp.asarray(gate_w)
            router_logits = xj @ gj.T
            probs = jax.nn.softmax(router_logits.astype(jnp.float32), axis=-1)
            rw, selected = jax.lax.top_k(probs, TOPK)
            rw = rw / jnp.sum(rw, axis=-1, keepdims=True)
            return (
                np.asarray(router_logits, dtype=np.float32),
                np.asarray(rw, dtype=np.float32),
                np.asarray(selected),
            )
    except Exception:
        router_logits = (x @ gate_w.T).astype(np.float32)
        z = router_logits - router_logits.max(axis=-1, keepdims=True)
        ez = np.exp(z, dtype=np.float32)
        probs = ez / ez.sum(axis=-1, keepdims=True)
        i1 = probs.argmax(axis=-1)
        p2 = probs.copy()
        p2[np.arange(T), i1] = -np.inf
        i2 = p2.argmax(axis=-1)
        selected = np.stack([i1, i2], axis=1)
        rw = np.take_along_axis(probs, selected, axis=1)
        rw = (rw / rw.sum(axis=-1, keepdims=True)).astype(np.float32)
        return router_logits, rw, selected


def _silu32(u: np.ndarray) -> np.ndarray:
    return (u / (1.0 + np.exp(-u))).astype(np.float32)


def kernel(hidden_states, gate_w, w1, w3, w2):
    import time as _time

    from concourse.bass_utils import run_bass_kernel_spmd

    _tlog = []
    _t0 = _time.time()

    hidden_states = np.asarray(hidden_states, dtype=np.float32)
    gate_w = np.asarray(gate_w, dtype=np.float32)
    w1 = np.asarray(w1, dtype=np.float32)
    w3 = np.asarray(w3, dtype=np.float32)
    w2 = np.asarray(w2, dtype=np.float32)

    x = hidden_states.reshape(T, H)
    router_logits, rw, selected = _host_routing(x, gate_w)
    _tlog.append(("routing", _time.time() - _t0)); _t0 = _time.time()

    # per-expert token lists + weights
    in_maps = []
    dev_idx = []
    host_work = []  # (e, idx, weights)
    for e in range(E):
        mask = selected == e  # [T, K]
        tok = np.nonzero(mask.any(axis=1))[0]
        wgt = (rw * mask).sum(axis=1)[tok].astype(np.float32)
        if len(tok) > C:
            host_work.append((e, tok[C:], wgt[C:]))
            tok, wgt = tok[:C], wgt[:C]
        dev_idx.append(tok)

        n = len(tok)
        xg = np.zeros((C, H), dtype=np.float32)
        xg[:n] = x[tok]
        sv = np.zeros((C,), dtype=np.float32)
        sv[:n] = wgt
        in_maps.append({
            "xT": np.ascontiguousarray(xg.T, dtype=np.float16),
            "w1T": np.ascontiguousarray(w1[e].T, dtype=np.float16),
            "w3T": np.ascontiguousarray(w3[e].T, dtype=np.float16),
            "w2T": np.ascontiguousarray(w2[e].T, dtype=np.float16),
            "sb": np.ascontiguousarray(
                np.broadcast_to(sv[None, :], (128, C))
            ),
        })

    _tlog.append(("prep", _time.time() - _t0)); _t0 = _time.time()

    nc = _get_program()
    _tlog.append(("build", _time.time() - _t0)); _t0 = _time.time()
    trace = bool(int(os.environ.get("MOE_KERNEL_TRACE", "0")))
    res = run_bass_kernel_spmd(nc, in_maps, list(range(8)), trace=trace)
    if trace:
        kernel.last_exec_time_ns = res.exec_time_ns
        kernel.last_results = res
    _tlog.append(("device", _time.time() - _t0)); _t0 = _time.time()

    out_T = np.zeros((H, T), dtype=np.float32)
    for e in range(E):
        tok = dev_idx[e]
        out_T[:, tok] += res.results[e]["yT"][:, :len(tok)]

    for e, tok, wgt in host_work:
        xo = x[tok]
        g = _silu32(xo @ w1[e].T) * (xo @ w3[e].T)
        out_T[:, tok] += (wgt[:, None] * (g @ w2[e].T)).T

    out = np.ascontiguousarray(out_T.T).reshape(B, S, H)
    _tlog.append(("combine", _time.time() - _t0))
    if os.environ.get("MOE_KERNEL_TIME"):
        print("kernel wall:", " ".join(f"{k}={v:.2f}s" for k, v in _tlog))
    return out, router_logits
